# revision 19
# baseline (speedup 1.0000x reference)
"""Trainium2 Bass kernel for nn_DirectionalWedgeBias.

Computes, per (batch b, head h):
    v      = x[b].reshape(T, H, Dh)[:, h, :]          # [T, Dh]
    v_hat  = v / max(||v||_2, eps)  (row-wise)
    S      = A[h] - A[h]^T                            # [Dh, Dh]
    wedge  = (v_hat @ S) @ v_hat^T                    # [T, T]

Full shapes: x [2, 2048, 1024] f32, A [16, 64, 64] f32 -> out [2, 16, 2048, 2048] f32.

Sharding: 32 independent (b, h) pairs split 4-per-core across 8 NeuronCores
(data + head parallel; the tiny skew-symmetric S is replicated/sliced with the
heads). Host pre-slices x into per-core [4, T, Dh] blocks, forms S = A - A^T,
and re-stacks the per-core [4, T, T] results.

Per-core dataflow (Tile framework), fp16 end to end (rel err ~5e-4 vs the
2e-2 gate; f32 is kept only for the row norms):
  - x loads use a 2-rows-per-partition interleave (t = n*256 + 2*part + r) so
    each partition receives contiguous 512 B runs (full-rate descriptors
    instead of the <512 B read-modify-write class)
  - row-normalize per 512-row group (DVE square+reduce f32, ACT sqrt, DVE
    reciprocal, broadcast multiply casting to fp16) so the first transposes
    start without waiting for the whole pair
  - PE-transpose v_hat chunks into vt [64, T] fp16 (true t order, stride-2
    column scatter on the PSUM evacuation); SvT = S @ vT is scattered
    block-major so wedge lhsT slices stay contiguous
  - wedge: one fp16 matmul per [128, 512] PSUM tile in a 6-deep ring (6 of
    the 8 banks), so the ring never throttles PE on evacuation latency;
    PSUM->SBUF evacuation casts to fp16 and rotates over ACT/DVE/Pool (the
    three engines that can read PSUM), never repeating an engine twice in a
    row; each [128, 1024] half-row chunk is stored as its own fp16 DMA
  - stores are fp16 (half the f32 DMA bytes) on the three DMA queues
    SP/Pool/ACT: a chunk's store chains in order behind its own second-half
    evacuation engine, or goes to SP through a one-chunk deferral queue --
    both avoid head-of-line blocking an engine queue on a cross-engine
    dependency; the host widens fp16 back to f32
  - a greedy balancer assigns every evacuation/store by modeled ns cost
  - the final n-block evacuates and stores at 512 grain across all queues to
    collapse the pipeline drain
  - walrus encodes at most ONE semaphore wait on most instructions, so
    `_spill_waits` post-processes the Tile-scheduled BIR, hoisting excess
    waits onto preceding same-engine EventSemaphores

Cost-model (CoreSim) engine-busy: ACT/SP/Pool/DVE ~67-73 us each, PE ~63 us.
"""

import numpy as np

B = 2
T = 2048
D = 1024
H = 16
Dh = 64
N_CORES = 8
PAIRS = (B * H) // N_CORES  # 4 per core
P = 128  # SBUF partitions
NB = T // 512  # 4 n-blocks (512 rows) per pair
R = 4  # t-rows per partition within an n-block

_COMPILED = {}

# test-harness knobs (default off; harness calls kernel() with these untouched)
TRACE = False
LAST_RESULT = None

# modeled per-instruction costs (ns) for the greedy engine balancer
_EVAC_512 = {"ACT": 670.0, "DVE": 728.0, "POOL": 520.0}
_STORE_Q = 850.0  # [128, 1024] fp16 chunk = 2048 B/partition
_LOAD_PAIR = 1729.0
# wedge evac engine rotation: None -> no-repeat greedy; else a static cycle
EVAC_PATTERN = None
_NORM_GROUP = 820.0  # vsq + reduce per 512-row group ([128, 4, 64]) on DVE
_VHAT_GROUP = 460.0


def _build_nc(pairs=PAIRS, t=T, spill=True):
    _import_concourse()
    from contextlib import ExitStack

    import concourse.bass as bass
    import concourse.tile as tile
    from concourse import mybir

    f32 = mybir.dt.float32
    f16 = mybir.dt.float16
    nb = t // 512  # n-blocks per pair
    ng = t // 512  # 512-row load/norm groups per pair (== nb for R=4)

    nc = bass.Bass()
    x_in = nc.declare_dram_parameter("x", [pairs, t, Dh], f32, isOutput=False)
    s_in = nc.declare_dram_parameter("s", [pairs, Dh, Dh], f32, isOutput=False)
    id_in = nc.declare_dram_parameter("ident", [P, P], f32, isOutput=False)
    out_d = nc.declare_dram_parameter("out", [pairs, t, t], f16, isOutput=True)

    busy = {"ACT": 0.0, "DVE": 0.0, "POOL": 0.0, "SP": 0.0}
    last_ev = {"e": None, "i": 0}

    def pick(cands, costs):
        k = min(cands, key=lambda e: busy[e] + costs[e])
        busy[k] += costs[k]
        return k

    def pick_ev(costs):
        # balanced evac choice that never repeats the previous engine, so
        # consecutive wedge-PSUM ring slots drain through different queues
        if EVAC_PATTERN is not None:
            k = EVAC_PATTERN[last_ev["i"] % len(EVAC_PATTERN)]
            last_ev["i"] += 1
            busy[k] += costs[k]
            return k
        cands = [e for e in ("ACT", "DVE", "POOL") if e != last_ev["e"]]
        k = min(cands, key=lambda e: busy[e] + costs[e])
        busy[k] += costs[k]
        last_ev["e"] = k
        return k

    def charge(eng_name, cost):
        busy[eng_name] += cost

    with ExitStack() as ctx:
        tc = ctx.enter_context(tile.TileContext(nc))
        eng = {"ACT": nc.scalar, "DVE": nc.vector, "POOL": nc.gpsimd, "SP": nc.sync}

        def copy_on(e, out, in_):
            if e == "ACT":
                eng[e].copy(out, in_)
            else:
                eng[e].tensor_copy(out, in_)

        const_pool = ctx.enter_context(tc.tile_pool(name="const", bufs=1))
        stage_pool = ctx.enter_context(tc.tile_pool(name="stage", bufs=2))
        pair_pool = ctx.enter_context(tc.tile_pool(name="pair", bufs=2))
        norm_pool = ctx.enter_context(tc.tile_pool(name="norm", bufs=2))
        psw_pool = ctx.enter_context(tc.tile_pool(name="psw", bufs=6, space="PSUM"))
        pvt_pool = ctx.enter_context(tc.tile_pool(name="pvt", bufs=1, space="PSUM"))
        psv_pool = ctx.enter_context(tc.tile_pool(name="psv", bufs=1, space="PSUM"))
        out_pool = ctx.enter_context(tc.tile_pool(name="outb", bufs=6))

        # identity: DMA-landed, staged through ACT (cast to fp16) so matmuls
        # only wait on ACT
        id_dma = const_pool.tile([P, P], f32)
        nc.sync.dma_start(out=id_dma, in_=id_in[:, :])
        identity = const_pool.tile([P, P], f16)
        nc.scalar.copy(identity, id_dma)
        charge("ACT", 300.0)
        # warmup matmuls: absorb the ACT(identity) wait and hold the PE
        # p-state ramp until the first real transposes arrive
        ps_warm = psv_pool.tile([Dh, 512], f32, tag="psv")
        for _ in range(10):
            nc.tensor.matmul(
                ps_warm[:, :P],
                lhsT=identity[:, :Dh],
                rhs=identity,
                start=True,
                stop=True,
            )

        gc = (nb * R) // ng  # (n, r) chunks per 512-row group = 4
        state = {}  # per-pair tiles
        sp_defer = []  # one-chunk SP store deferral against HOL blocking

        def prep_load_norm(p):
            """Load x[p] (512 B runs per partition), row-normalize, cast fp16."""
            ctx_lp = nc.allow_low_precision(
                reason="unit-variance 64-dim row norms fit fp16 comfortably"
            )
            ctx_lp.__enter__()
            s_dma = stage_pool.tile([Dh, Dh], f32, tag="sdma")
            nc.scalar.dma_start(out=s_dma, in_=s_in[p])
            charge("ACT", 500.0)
            s_sb = pair_pool.tile([Dh, Dh], f16, tag="s")
            nc.scalar.copy(s_sb[:], s_dma)
            charge("ACT", 240.0)

            v_sb = pair_pool.tile([P, nb * R, Dh], f16, tag="v")
            vsq = norm_pool.tile([P, nb * R, Dh], f16, tag="vsq")
            sumsq = norm_pool.tile([P, nb * R], f16, tag="ss")
            if p > 0:
                # gpsimd casting DMA: f32 DRAM -> fp16 SBUF at half the bytes
                nc.gpsimd.dma_start(
                    out=v_sb[:].rearrange("p (n r) d -> p n r d", r=R),
                    in_=x_in[p].rearrange("(n p r) d -> p n r d", p=P, r=R),
                )
                charge("POOL", _LOAD_PAIR / 2 + 100.0)
            for g in range(ng):
                if p == 0:
                    nc.gpsimd.dma_start(
                        out=v_sb[:, g * gc : (g + 1) * gc, :].rearrange(
                            "p (n r) d -> p n r d", r=R
                        ),
                        in_=x_in[p][g * 512 : (g + 1) * 512, :].rearrange(
                            "(n p r) d -> p n r d", p=P, r=R
                        ),
                    )
                    charge("POOL", 500.0)
                nc.vector.tensor_mul(
                    vsq[:, g * gc : (g + 1) * gc, :],
                    v_sb[:, g * gc : (g + 1) * gc, :],
                    v_sb[:, g * gc : (g + 1) * gc, :],
                )
                nc.vector.reduce_sum(
                    sumsq[:, g * gc : (g + 1) * gc],
                    vsq[:, g * gc : (g + 1) * gc, :],
                    axis=mybir.AxisListType.X,
                )
                charge("DVE", _NORM_GROUP)
            nrm = norm_pool.tile([P, nb * R], f16, tag="nrm")
            rinv = norm_pool.tile([P, nb * R], f16, tag="rinv")
            v_hat = pair_pool.tile([P, nb * R, Dh], f16, tag="vhat")
            for g in range(ng):
                sl = slice(g * gc, (g + 1) * gc)
                nc.scalar.activation(
                    nrm[:, sl], sumsq[:, sl], mybir.ActivationFunctionType.Sqrt
                )
                charge("ACT", 200.0)
                nc.vector.reciprocal(rinv[:, sl], nrm[:, sl])
                charge("DVE", 120.0)
                rb = rinv[:, sl].unsqueeze(-1).broadcast_to((P, gc, Dh))
                nc.vector.tensor_mul(v_hat[:, sl, :], v_sb[:, sl, :], rb)
                charge("DVE", _VHAT_GROUP)
            ctx_lp.__exit__(None, None, None)
            vt_sb = pair_pool.tile([Dh, t], f16, tag="vt")
            svt_sb = pair_pool.tile([Dh, t], f16, tag="svt")
            state[p] = {"s": s_sb, "vhat": v_hat, "vt": vt_sb, "svt": svt_sb}

        def prep_pe_group(p, g):
            """PE-transpose group g of v_hat into vt (true t order) and form
            that group's SvT slice (block-major)."""
            st = state[p]
            ps_vt = pvt_pool.tile([Dh, 512], f16, tag="pvt")
            for j in range(gc):
                nc.tensor.transpose(
                    ps_vt[:, j * P : (j + 1) * P],
                    st["vhat"][:, g * gc + j, :],
                    identity,
                )
            # ps_vt is block-ordered [(n, r) chunks, j]; chunk (n, r) holds
            # t = n*256 + 2*j + r -> scatter into true-t-order vt
            e = pick(("ACT", "DVE", "POOL"), _EVAC_512)
            copy_on(
                e,
                st["vt"][:, g * 512 : (g + 1) * 512].rearrange(
                    "d (n j r) -> d n r j", n=gc // R, j=P, r=R
                ),
                ps_vt.rearrange("d (n r j) -> d n r j", n=gc // R, r=R, j=P),
            )
            ps_sv = psv_pool.tile([Dh, 512], f32, tag="psv")
            nc.tensor.matmul(
                ps_sv,
                lhsT=st["s"][:],
                rhs=st["vt"][:, g * 512 : (g + 1) * 512],
                start=True,
                stop=True,
            )
            # ps_sv is true-t-ordered; scatter block-major so wedge lhsT
            # slices are contiguous: svt col (n*R + r)*128 + j <- t
            e = pick(("ACT", "DVE", "POOL"), _EVAC_512)
            copy_on(
                e,
                st["svt"][:, g * 512 : (g + 1) * 512].rearrange(
                    "d (n r j) -> d n j r", n=gc // R, r=R, j=P
                ),
                ps_sv.rearrange("d (n j r) -> d n j r", n=gc // R, j=P, r=R),
            )

        def wedge_block(p, n):
            """One 256-row n-block: 8 [128, 512] PSUM ring tiles; each is
            evacuated (fp16 cast) on a rotating engine; every [128, 1024]
            chunk is stored as its own quarter DMA."""
            st = state[p]
            ob = out_pool.tile([P, R, t], f16, tag="ob")
            dst = out_d[p][n * 512 : (n + 1) * 512, :].rearrange(
                "(j r) c -> j r c", r=R
            )
            store = {k: _STORE_Q for k in ("SP", "POOL", "ACT")}
            fine = p == pairs - 1 and n == nb - 1
            for r in range(R):
                blk = n * R + r
                for h in range(t // 1024):
                    evs = []
                    for u in range(2):
                        ps_w = psw_pool.tile([P, 512], f32, tag="psw")
                        nc.tensor.matmul(
                            ps_w,
                            lhsT=st["svt"][:, blk * P : (blk + 1) * P],
                            rhs=st["vt"][
                                :, (h * 2 + u) * 512 : (h * 2 + u + 1) * 512
                            ],
                            start=True,
                            stop=True,
                        )
                        lo = h * 1024 + u * 512
                        ev = pick_ev(_EVAC_512)
                        evs.append(ev)
                        copy_on(ev, ob[:, r, lo : lo + 512], ps_w)
                        if fine:
                            # pipeline drain: store the tail at 512 grain
                            cands = ("SP", ev) if ev != "DVE" else ("SP",)
                            e = pick(cands, {k: 500.0 for k in cands})
                            eng[e].dma_start(
                                out=dst[:, r, lo : lo + 512],
                                in_=ob[:, r, lo : lo + 512],
                            )
                    if fine:
                        continue
                    # store chains behind the second evac's engine (its first
                    # evac is already ahead in a parallel queue) or goes to SP
                    # via a one-chunk deferral -- never a third engine, which
                    # would head-of-line block on a cross-engine dependency
                    ev = evs[-1]
                    cands = ("SP", ev) if ev != "DVE" else ("SP",)
                    e = pick(cands, store)
                    if e == "SP":
                        sp_defer.append(
                            (
                                dst[:, r, h * 1024 : (h + 1) * 1024],
                                ob[:, r, h * 1024 : (h + 1) * 1024],
                            )
                        )
                        if len(sp_defer) > 1:
                            o_, i_ = sp_defer.pop(0)
                            nc.sync.dma_start(out=o_, in_=i_)
                    else:
                        eng[e].dma_start(
                            out=dst[:, r, h * 1024 : (h + 1) * 1024],
                            in_=ob[:, r, h * 1024 : (h + 1) * 1024],
                        )

        # software pipeline: pair p's wedge overlaps pair p+1's load/norm
        # (emitted first so DVE runs it early) and its transpose/Sv groups
        # (interleaved mid-wedge so the evacs drain before the wedge tail)
        prep_load_norm(0)
        for g in range(ng):
            prep_pe_group(0, g)
        for p in range(pairs):
            if p + 1 < pairs:
                prep_load_norm(p + 1)
            for n in range(nb):
                if p == pairs - 1 and n == nb - 1:
                    for o_, i_ in sp_defer:
                        nc.sync.dma_start(out=o_, in_=i_)
                    sp_defer.clear()
                wedge_block(p, n)
                if p + 1 < pairs:
                    prep_pe_group(p + 1, n)
            state.pop(p)
        for o_, i_ in sp_defer:
            nc.sync.dma_start(out=o_, in_=i_)

    if spill:
        _spill_waits(nc)
    nc._balancer_busy = dict(busy)
    return nc


def _spill_waits(nc, multi_ok=("EventSemaphore",), max_keep=1):
    """Walrus encodes at most one sync-wait on Matmult (embedded weight load)
    and DMACopy; move extra waits onto a preceding same-engine EventSemaphore
    (which supports many waits). The engine sequencer processes instructions
    in order, so a preceding wait is semantically identical."""
    from concourse import mybir

    n_spilled = 0
    for f in nc.m.functions:
        for bb in f.blocks:
            il = bb.instructions
            out = []
            for inst in il:
                si = getattr(inst, "sync_info", None)
                waits = list((si.on_wait if si else None) or [])
                cap = 2 if inst.opcode in multi_ok else max_keep
                if len(waits) > cap:
                    moved, keep = waits[:-max_keep], waits[-max_keep:]
                    for k in range(0, len(moved), 2):
                        es = mybir.InstEventSemaphore(
                            name=f"{inst.name}-wspill{k}",
                            engine=inst.engine,
                            ins=[],
                            outs=[],
                            sync_info=mybir.SyncInfo(
                                on_wait=moved[k : k + 2], on_update=[]
                            ),
                        )
                        out.append(es)
                    inst.sync_info = mybir.SyncInfo(
                        on_wait=keep, on_update=list(si.on_update or [])
                    )
                    n_spilled += 1
                out.append(inst)
            il[:] = out
    return n_spilled


def _import_concourse():
    try:
        import concourse  # noqa: F401
    except ImportError:
        import sys

        for p in ("/opt/trn_rl_repo", "/root/.axon_site/_ro/trn_rl_repo"):
            if p not in sys.path:
                sys.path.insert(0, p)


def _ensure_device_backend():
    """If the process pinned JAX_PLATFORMS to cpu, lift the pin so the
    NeuronCores (axon platform) are reachable for the kernel run."""
    import os

    plats = os.environ.get("JAX_PLATFORMS", "")
    if plats and "axon" not in plats and "neuron" not in plats:
        os.environ["JAX_PLATFORMS"] = ""
        try:
            import jax

            jax.extend.backend.clear_backends()
        except Exception:
            pass


def kernel(x, A, window_size=None):
    _import_concourse()
    _ensure_device_backend()
    from concourse.bass_utils import run_bass_kernel_spmd

    x = np.ascontiguousarray(x, dtype=np.float32)
    A = np.ascontiguousarray(A, dtype=np.float32)
    assert x.shape == (B, T, D) and A.shape == (H, Dh, Dh)

    nc = _COMPILED.get("nc")
    if nc is None:
        nc = _build_nc()
        _COMPILED["nc"] = nc

    # x[b, t, h*64:(h+1)*64] per (b,h) pair; pair index bh = b*H + h.
    xv = x.reshape(B, T, H, Dh).transpose(0, 2, 1, 3).reshape(B * H, T, Dh)
    S = (A - np.swapaxes(A, -1, -2)).astype(np.float32)  # replicated with heads
    S_all = np.tile(S, (B, 1, 1))
    ident = np.eye(P, dtype=np.float32)
    in_maps = []
    for c in range(N_CORES):
        sl = slice(c * PAIRS, (c + 1) * PAIRS)
        in_maps.append(
            {
                "x": np.ascontiguousarray(xv[sl]),
                "s": np.ascontiguousarray(S_all[sl]),
                "ident": ident,
            }
        )
    res = run_bass_kernel_spmd(nc, in_maps, list(range(N_CORES)), trace=TRACE)
    global LAST_RESULT
    LAST_RESULT = res
    outs = [np.asarray(res.results[c]["out"]) for c in range(N_CORES)]
    full = np.concatenate(outs, axis=0).reshape(B, H, T, T).astype(np.float32)
    return full


# revision 20
# speedup vs baseline: 1.0613x; 1.0613x over previous
"""Trainium2 Bass kernel for nn_DirectionalWedgeBias.

Computes, per (batch b, head h):
    v      = x[b].reshape(T, H, Dh)[:, h, :]          # [T, Dh]
    v_hat  = v / max(||v||_2, eps)  (row-wise)
    S      = A[h] - A[h]^T                            # [Dh, Dh]
    wedge  = (v_hat @ S) @ v_hat^T                    # [T, T]

Full shapes: x [2, 2048, 1024] f32, A [16, 64, 64] f32 -> out [2, 16, 2048, 2048] f32.

Sharding: 32 independent (b, h) pairs split 4-per-core across 8 NeuronCores
(data + head parallel; the tiny skew-symmetric S is replicated/sliced with the
heads). Host pre-slices x into per-core [4, T, Dh] blocks, forms S = A - A^T,
and re-stacks the per-core [4, T, T] results.

Per-core dataflow (Tile framework), fp16 end to end (rel err ~5e-4 vs the
2e-2 gate; f32 is kept only for the row norms):
  - x loads use a 2-rows-per-partition interleave (t = n*256 + 2*part + r) so
    each partition receives contiguous 512 B runs (full-rate descriptors
    instead of the <512 B read-modify-write class)
  - row-normalize per 512-row group (DVE square+reduce f32, ACT sqrt, DVE
    reciprocal, broadcast multiply casting to fp16) so the first transposes
    start without waiting for the whole pair
  - PE-transpose v_hat chunks into vt [64, T] fp16 (true t order, stride-2
    column scatter on the PSUM evacuation); SvT = S @ vT is scattered
    block-major so wedge lhsT slices stay contiguous
  - wedge: one fp16 matmul per [128, 512] PSUM tile in a 6-deep ring (6 of
    the 8 banks), so the ring never throttles PE on evacuation latency;
    PSUM->SBUF evacuation casts to fp16 and rotates over ACT/DVE/Pool (the
    three engines that can read PSUM), never repeating an engine twice in a
    row; each [128, 1024] half-row chunk is stored as its own fp16 DMA
  - stores are fp16 (half the f32 DMA bytes) on the three DMA queues
    SP/Pool/ACT: a chunk's store chains in order behind its own second-half
    evacuation engine, or goes to SP through a one-chunk deferral queue --
    both avoid head-of-line blocking an engine queue on a cross-engine
    dependency; the host widens fp16 back to f32
  - a greedy balancer assigns every evacuation/store by modeled ns cost
  - the final n-block evacuates and stores at 512 grain across all queues to
    collapse the pipeline drain
  - walrus encodes at most ONE semaphore wait on most instructions, so
    `_spill_waits` post-processes the Tile-scheduled BIR, hoisting excess
    waits onto preceding same-engine EventSemaphores

Cost-model (CoreSim) engine-busy: ACT/SP/Pool/DVE ~67-73 us each, PE ~63 us.
"""

import numpy as np

B = 2
T = 2048
D = 1024
H = 16
Dh = 64
N_CORES = 8
PAIRS = (B * H) // N_CORES  # 4 per core
P = 128  # SBUF partitions
NB = T // 256  # 8 n-blocks (256 rows) per pair
R = 2  # t-rows per partition within an n-block

_COMPILED = {}

# test-harness knobs (default off; harness calls kernel() with these untouched)
TRACE = False
LAST_RESULT = None

# modeled per-instruction costs (ns) for the greedy engine balancer
_EVAC_512 = {"ACT": 670.0, "DVE": 728.0, "POOL": 520.0}
_STORE_Q = 850.0  # [128, 1024] fp16 chunk = 2048 B/partition
_LOAD_PAIR = 1729.0
# wedge evac engine rotation: None -> no-repeat greedy; else a static cycle
EVAC_PATTERN = None
_NORM_GROUP = 820.0  # vsq + reduce per 512-row group ([128, 4, 64]) on DVE
_VHAT_GROUP = 460.0


def _build_nc(pairs=PAIRS, t=T, spill=True):
    _import_concourse()
    from contextlib import ExitStack

    import concourse.bass as bass
    import concourse.tile as tile
    from concourse import mybir

    f32 = mybir.dt.float32
    f16 = mybir.dt.float16
    nb = t // 256  # n-blocks per pair
    ng = t // 512  # 512-row load/norm groups per pair

    nc = bass.Bass()
    x_in = nc.declare_dram_parameter("x", [pairs, t, Dh], f32, isOutput=False)
    s_in = nc.declare_dram_parameter("s", [pairs, Dh, Dh], f32, isOutput=False)
    id_in = nc.declare_dram_parameter("ident", [P, P], f32, isOutput=False)
    out_d = nc.declare_dram_parameter("out", [pairs, t, t], f16, isOutput=True)

    busy = {"ACT": 0.0, "DVE": 0.0, "POOL": 0.0, "SP": 0.0}
    last_ev = {"e": None, "i": 0}

    def pick(cands, costs):
        k = min(cands, key=lambda e: busy[e] + costs[e])
        busy[k] += costs[k]
        return k

    def pick_ev(costs):
        # balanced evac choice that never repeats the previous engine, so
        # consecutive wedge-PSUM ring slots drain through different queues
        if EVAC_PATTERN is not None:
            k = EVAC_PATTERN[last_ev["i"] % len(EVAC_PATTERN)]
            last_ev["i"] += 1
            busy[k] += costs[k]
            return k
        cands = [e for e in ("ACT", "DVE", "POOL") if e != last_ev["e"]]
        k = min(cands, key=lambda e: busy[e] + costs[e])
        busy[k] += costs[k]
        last_ev["e"] = k
        return k

    def charge(eng_name, cost):
        busy[eng_name] += cost

    with ExitStack() as ctx:
        tc = ctx.enter_context(tile.TileContext(nc))
        eng = {"ACT": nc.scalar, "DVE": nc.vector, "POOL": nc.gpsimd, "SP": nc.sync}

        def copy_on(e, out, in_):
            if e == "ACT":
                eng[e].copy(out, in_)
            else:
                eng[e].tensor_copy(out, in_)

        const_pool = ctx.enter_context(tc.tile_pool(name="const", bufs=1))
        stage_pool = ctx.enter_context(tc.tile_pool(name="stage", bufs=2))
        pair_pool = ctx.enter_context(tc.tile_pool(name="pair", bufs=2))
        norm_pool = ctx.enter_context(tc.tile_pool(name="norm", bufs=2))
        psw_pool = ctx.enter_context(tc.tile_pool(name="psw", bufs=6, space="PSUM"))
        pvt_pool = ctx.enter_context(tc.tile_pool(name="pvt", bufs=1, space="PSUM"))
        psv_pool = ctx.enter_context(tc.tile_pool(name="psv", bufs=1, space="PSUM"))
        out_pool = ctx.enter_context(tc.tile_pool(name="outb", bufs=6))

        # identity: DMA-landed, staged through ACT (cast to fp16) so matmuls
        # only wait on ACT
        id_dma = const_pool.tile([P, P], f32)
        nc.sync.dma_start(out=id_dma, in_=id_in[:, :])
        identity = const_pool.tile([P, P], f16)
        nc.scalar.copy(identity, id_dma)
        charge("ACT", 300.0)
        # warmup matmuls: absorb the ACT(identity) wait and hold the PE
        # p-state ramp until the first real transposes arrive
        ps_warm = psv_pool.tile([Dh, 512], f32, tag="psv")
        for _ in range(10):
            nc.tensor.matmul(
                ps_warm[:, :P],
                lhsT=identity[:, :Dh],
                rhs=identity,
                start=True,
                stop=True,
            )

        gc = (nb * R) // ng  # (n, r) chunks per 512-row group = 4
        state = {}  # per-pair tiles
        sp_defer = []  # one-chunk SP store deferral against HOL blocking

        def prep_load_norm(p):
            """Load x[p] (512 B runs per partition), row-normalize, cast fp16."""
            s_dma = stage_pool.tile([Dh, Dh], f32, tag="sdma")
            nc.scalar.dma_start(out=s_dma, in_=s_in[p])
            charge("ACT", 500.0)
            s_sb = pair_pool.tile([Dh, Dh], f16, tag="s")
            nc.scalar.copy(s_sb[:], s_dma)
            charge("ACT", 240.0)

            v_sb = pair_pool.tile([P, nb * R, Dh], f32, tag="v")
            vsq = norm_pool.tile([P, nb * R, Dh], f32, tag="vsq")
            sumsq = norm_pool.tile([P, nb * R], f32, tag="ss")
            if p > 0:
                nc.sync.dma_start(
                    out=v_sb[:].rearrange("p (n r) d -> p n r d", r=R),
                    in_=x_in[p].rearrange("(n p r) d -> p n r d", p=P, r=R),
                )
                charge("SP", _LOAD_PAIR + 150.0)
            for g in range(ng):
                if p == 0:
                    ld = ("SP", "POOL", "ACT", "POOL")[g % 4]
                    eng[ld].dma_start(
                        out=v_sb[:, g * gc : (g + 1) * gc, :].rearrange(
                            "p (n r) d -> p n r d", r=R
                        ),
                        in_=x_in[p][g * 512 : (g + 1) * 512, :].rearrange(
                            "(n p r) d -> p n r d", p=P, r=R
                        ),
                    )
                    charge(ld, _LOAD_PAIR / ng + 150.0)
                nc.vector.tensor_mul(
                    vsq[:, g * gc : (g + 1) * gc, :],
                    v_sb[:, g * gc : (g + 1) * gc, :],
                    v_sb[:, g * gc : (g + 1) * gc, :],
                )
                nc.vector.reduce_sum(
                    sumsq[:, g * gc : (g + 1) * gc],
                    vsq[:, g * gc : (g + 1) * gc, :],
                    axis=mybir.AxisListType.X,
                )
                charge("DVE", _NORM_GROUP)
            nrm = norm_pool.tile([P, nb * R], f32, tag="nrm")
            rinv = norm_pool.tile([P, nb * R], f32, tag="rinv")
            v_hat = pair_pool.tile([P, nb * R, Dh], f16, tag="vhat")
            for g in range(ng):
                sl = slice(g * gc, (g + 1) * gc)
                nc.scalar.activation(
                    nrm[:, sl], sumsq[:, sl], mybir.ActivationFunctionType.Sqrt
                )
                charge("ACT", 200.0)
                nc.vector.reciprocal(rinv[:, sl], nrm[:, sl])
                charge("DVE", 120.0)
                rb = rinv[:, sl].unsqueeze(-1).broadcast_to((P, gc, Dh))
                nc.vector.tensor_mul(v_hat[:, sl, :], v_sb[:, sl, :], rb)
                charge("DVE", _VHAT_GROUP)
            vt_sb = pair_pool.tile([Dh, t], f16, tag="vt")
            svt_sb = pair_pool.tile([Dh, t], f16, tag="svt")
            state[p] = {"s": s_sb, "vhat": v_hat, "vt": vt_sb, "svt": svt_sb}

        def prep_pe_group(p, g):
            """PE-transpose group g of v_hat into vt (true t order) and form
            that group's SvT slice (block-major)."""
            st = state[p]
            ps_vt = pvt_pool.tile([Dh, 512], f16, tag="pvt")
            for j in range(gc):
                nc.tensor.transpose(
                    ps_vt[:, j * P : (j + 1) * P],
                    st["vhat"][:, g * gc + j, :],
                    identity,
                )
            # ps_vt is block-ordered [(n, r) chunks, j]; chunk (n, r) holds
            # t = n*256 + 2*j + r -> scatter into true-t-order vt
            e = pick(("ACT", "DVE", "POOL"), _EVAC_512)
            copy_on(
                e,
                st["vt"][:, g * 512 : (g + 1) * 512].rearrange(
                    "d (n j r) -> d n r j", n=gc // R, j=P, r=R
                ),
                ps_vt.rearrange("d (n r j) -> d n r j", n=gc // R, r=R, j=P),
            )
            ps_sv = psv_pool.tile([Dh, 512], f32, tag="psv")
            nc.tensor.matmul(
                ps_sv,
                lhsT=st["s"][:],
                rhs=st["vt"][:, g * 512 : (g + 1) * 512],
                start=True,
                stop=True,
            )
            # ps_sv is true-t-ordered; scatter block-major so wedge lhsT
            # slices are contiguous: svt col (n*R + r)*128 + j <- t
            e = pick(("ACT", "DVE", "POOL"), _EVAC_512)
            copy_on(
                e,
                st["svt"][:, g * 512 : (g + 1) * 512].rearrange(
                    "d (n r j) -> d n j r", n=gc // R, r=R, j=P
                ),
                ps_sv.rearrange("d (n j r) -> d n j r", n=gc // R, j=P, r=R),
            )

        def wedge_block(p, n):
            """One 256-row n-block: 8 [128, 512] PSUM ring tiles; each is
            evacuated (fp16 cast) on a rotating engine; every [128, 1024]
            chunk is stored as its own quarter DMA."""
            st = state[p]
            ob = out_pool.tile([P, R, t], f16, tag="ob")
            dst = out_d[p][n * 256 : (n + 1) * 256, :].rearrange(
                "(j r) c -> j r c", r=R
            )
            store = {k: _STORE_Q for k in ("SP", "POOL", "ACT")}
            fine = p == pairs - 1 and n == nb - 1
            for r in range(R):
                blk = n * R + r
                for h in range(t // 1024):
                    evs = []
                    for u in range(2):
                        ps_w = psw_pool.tile([P, 512], f32, tag="psw")
                        nc.tensor.matmul(
                            ps_w,
                            lhsT=st["svt"][:, blk * P : (blk + 1) * P],
                            rhs=st["vt"][
                                :, (h * 2 + u) * 512 : (h * 2 + u + 1) * 512
                            ],
                            start=True,
                            stop=True,
                        )
                        lo = h * 1024 + u * 512
                        ev = pick_ev(_EVAC_512)
                        evs.append(ev)
                        copy_on(ev, ob[:, r, lo : lo + 512], ps_w)
                        if fine:
                            # pipeline drain: store the tail at 512 grain
                            cands = ("SP", ev) if ev != "DVE" else ("SP",)
                            e = pick(cands, {k: 500.0 for k in cands})
                            eng[e].dma_start(
                                out=dst[:, r, lo : lo + 512],
                                in_=ob[:, r, lo : lo + 512],
                            )
                    if fine:
                        continue
                    # store chains behind the second evac's engine (its first
                    # evac is already ahead in a parallel queue) or goes to SP
                    # via a one-chunk deferral -- never a third engine, which
                    # would head-of-line block on a cross-engine dependency
                    ev = evs[-1]
                    cands = ("SP", ev) if ev != "DVE" else ("SP",)
                    e = pick(cands, store)
                    if e == "SP":
                        sp_defer.append(
                            (
                                dst[:, r, h * 1024 : (h + 1) * 1024],
                                ob[:, r, h * 1024 : (h + 1) * 1024],
                            )
                        )
                        if len(sp_defer) > 1:
                            o_, i_ = sp_defer.pop(0)
                            nc.sync.dma_start(out=o_, in_=i_)
                    else:
                        eng[e].dma_start(
                            out=dst[:, r, h * 1024 : (h + 1) * 1024],
                            in_=ob[:, r, h * 1024 : (h + 1) * 1024],
                        )

        # software pipeline: pair p's wedge overlaps pair p+1's load/norm
        # (emitted first so DVE runs it early) and its transpose/Sv groups
        # (interleaved mid-wedge so the evacs drain before the wedge tail)
        prep_load_norm(0)
        for g in range(ng):
            prep_pe_group(0, g)
        for p in range(pairs):
            if p + 1 < pairs:
                prep_load_norm(p + 1)
            for n in range(nb):
                if p == pairs - 1 and n == nb - 1:
                    for o_, i_ in sp_defer:
                        nc.sync.dma_start(out=o_, in_=i_)
                    sp_defer.clear()
                wedge_block(p, n)
                if p + 1 < pairs and n - 3 in range(ng):
                    prep_pe_group(p + 1, n - 3)
            state.pop(p)
        for o_, i_ in sp_defer:
            nc.sync.dma_start(out=o_, in_=i_)

    if spill:
        _spill_waits(nc)
    nc._balancer_busy = dict(busy)
    return nc


def _spill_waits(nc, multi_ok=("EventSemaphore",), max_keep=1):
    """Walrus encodes at most one sync-wait on Matmult (embedded weight load)
    and DMACopy; move extra waits onto a preceding same-engine EventSemaphore
    (which supports many waits). The engine sequencer processes instructions
    in order, so a preceding wait is semantically identical."""
    from concourse import mybir

    n_spilled = 0
    for f in nc.m.functions:
        for bb in f.blocks:
            il = bb.instructions
            out = []
            for inst in il:
                si = getattr(inst, "sync_info", None)
                waits = list((si.on_wait if si else None) or [])
                cap = 2 if inst.opcode in multi_ok else max_keep
                if len(waits) > cap:
                    moved, keep = waits[:-max_keep], waits[-max_keep:]
                    for k in range(0, len(moved), 2):
                        es = mybir.InstEventSemaphore(
                            name=f"{inst.name}-wspill{k}",
                            engine=inst.engine,
                            ins=[],
                            outs=[],
                            sync_info=mybir.SyncInfo(
                                on_wait=moved[k : k + 2], on_update=[]
                            ),
                        )
                        out.append(es)
                    inst.sync_info = mybir.SyncInfo(
                        on_wait=keep, on_update=list(si.on_update or [])
                    )
                    n_spilled += 1
                out.append(inst)
            il[:] = out
    return n_spilled


def _import_concourse():
    try:
        import concourse  # noqa: F401
    except ImportError:
        import sys

        for p in ("/opt/trn_rl_repo", "/root/.axon_site/_ro/trn_rl_repo"):
            if p not in sys.path:
                sys.path.insert(0, p)


def _ensure_device_backend():
    """If the process pinned JAX_PLATFORMS to cpu, lift the pin so the
    NeuronCores (axon platform) are reachable for the kernel run."""
    import os

    plats = os.environ.get("JAX_PLATFORMS", "")
    if plats and "axon" not in plats and "neuron" not in plats:
        os.environ["JAX_PLATFORMS"] = ""
        try:
            import jax

            jax.extend.backend.clear_backends()
        except Exception:
            pass


def kernel(x, A, window_size=None):
    _import_concourse()
    _ensure_device_backend()
    from concourse.bass_utils import run_bass_kernel_spmd

    x = np.ascontiguousarray(x, dtype=np.float32)
    A = np.ascontiguousarray(A, dtype=np.float32)
    assert x.shape == (B, T, D) and A.shape == (H, Dh, Dh)

    nc = _COMPILED.get("nc")
    if nc is None:
        nc = _build_nc()
        _COMPILED["nc"] = nc

    # x[b, t, h*64:(h+1)*64] per (b,h) pair; pair index bh = b*H + h.
    xv = x.reshape(B, T, H, Dh).transpose(0, 2, 1, 3).reshape(B * H, T, Dh)
    S = (A - np.swapaxes(A, -1, -2)).astype(np.float32)  # replicated with heads
    S_all = np.tile(S, (B, 1, 1))
    ident = np.eye(P, dtype=np.float32)
    in_maps = []
    for c in range(N_CORES):
        sl = slice(c * PAIRS, (c + 1) * PAIRS)
        in_maps.append(
            {
                "x": np.ascontiguousarray(xv[sl]),
                "s": np.ascontiguousarray(S_all[sl]),
                "ident": ident,
            }
        )
    res = run_bass_kernel_spmd(nc, in_maps, list(range(N_CORES)), trace=TRACE)
    global LAST_RESULT
    LAST_RESULT = res
    outs = [np.asarray(res.results[c]["out"]) for c in range(N_CORES)]
    full = np.concatenate(outs, axis=0).reshape(B, H, T, T).astype(np.float32)
    return full
